# revision 1
# baseline (speedup 1.0000x reference)
"""Trainium2 Bass kernel for nn_EntityClassify (3-layer R-GCN over a
heterograph with node types a/b/d and 4 relations).

Strategy (8 NeuronCores, SPMD):
  - Dead-code pruning: the output is h3['d'], which only needs
        L0: h1_d = relu(mean_r0(feat_a) @ W0[0] + mean_r1(feat_b) @ W0[1] + b0)
        L1: t_a = relu(mean_r2(h1_d) @ W1[2] + b1) @ W2[0]
            t_b = relu(mean_r3(h1_d) @ W1[3] + b1) @ W2[1]
        L2: out  = mean_r0(t_a) + mean_r1(t_b) + b2
    (feat_d and the other relation weights are dead.)
  - Destination-node partitioning across 8 cores; edges bucketed on the host
    by (dst core, 256-row dst cell = 2 windows, src subtable-quarter), sorted,
    padded to a shared (max-over-cores) chunk grid.  256-wide cells halve the
    ceil-padding vs per-window bucketing (17% vs 34%).
  - Gathers are batched SWDGE dma_gathers (int16 indices, <=1024 descriptors
    each -- the descriptor-ring cap), spread round-robin over 4 SWDGE queues
    (parallel descriptor streams; per-descriptor fixed cost dominates).
  - Tables and masks are fp16 (PE 1 cycle/row); segment sums accumulate in
    fp32 PSUM via one-hot-mask matmuls over [128-edge x 256-dst] masks.
  - Between layers: AllGather of h1 (fp16 rows) and of the compact 16-wide
    t tables, which are then expanded locally (strided D2D DMA) into a
    256B-stride padded layout so L2 can dma_gather them.
"""

from contextlib import ExitStack

import numpy as np

P = 128
NCORES = 8
SUB = 25088  # subtable rows (int16-indexable); 4*SUB = 100352 = padded a/b space
WPP = 2      # windows per PSUM cell (256-wide masks)

CFG = dict(
    ND=50000, NA=100000, NB=100000,
    H=128, OUT=16, E=600000,
    DPC=6272,    # d-rows per core (49 windows, 25 cells)
    APC=12544,   # a/b-rows per core (98 windows, 49 cells)
    CG0=4,       # cells per gather-group, d-side passes (L0/L2)
    CG1=4,       # cells per gather-group, a/b-side passes (L1)
)


def _ceil_div(a, b):
    return -(-a // b)


def _prep_relation(src, dst, rpc, n_dst_real, n_sub, sub, cells_per_group,
                   n_cores=NCORES):
    """Bucket/sort edges by (dst core, dst cell, src quarter); build the
    shared chunk grid and per-core gather-index + mask-meta arrays."""
    src = np.asarray(src).astype(np.int64)
    dst = np.asarray(dst).astype(np.int64)
    deg = np.bincount(dst, minlength=n_dst_real)
    wnode = (1.0 / np.maximum(deg, 1.0)).astype(np.float32)

    n_win = rpc // P
    n_cell = _ceil_div(n_win, WPP)
    CW = P * WPP
    core = dst // rpc
    rem = dst % rpc
    cell = rem // CW
    col = rem % CW
    q = src // sub
    lidx = src % sub
    assert q.max() < n_sub

    key = (core * n_cell + cell) * n_sub + q
    order = np.argsort(key, kind="stable")
    s_key = key[order]
    s_core = core[order]
    s_cell = cell[order]
    s_q = q[order]
    s_col = col[order]
    s_w = wnode[dst[order]]
    s_lidx = lidx[order]

    counts = np.bincount(s_key, minlength=n_cores * n_cell * n_sub).reshape(
        n_cores, n_cell, n_sub
    )
    K = _ceil_div(counts.max(axis=0), P)  # [n_cell, n_sub]
    K[:, 0] = np.maximum(K[:, 0], 1)

    groups = [
        list(range(g0, min(g0 + cells_per_group, n_cell)))
        for g0 in range(0, n_cell, cells_per_group)
    ]
    chunk_base = np.zeros((n_cell, n_sub), np.int64)
    spans = {}          # (g, q) -> (chunk0, nchunks)
    group_span = []     # g -> (chunk0, nchunks)
    cb = 0
    for g, cs_ in enumerate(groups):
        g0 = cb
        for q_ in range(n_sub):
            c0 = cb
            for ce in cs_:
                chunk_base[ce, q_] = cb
                cb += K[ce, q_]
            spans[(g, q_)] = (c0, cb - c0)
        group_span.append((g0, cb - g0))
    T = int(cb)

    starts = np.zeros(n_cores * n_cell * n_sub + 1, np.int64)
    starts[1:] = np.cumsum(counts.reshape(-1))
    rank = np.arange(len(dst), dtype=np.int64) - starts[s_key]
    pos = chunk_base[s_cell, s_q] * P + rank

    idxA = np.zeros((n_cores, T * P), np.int16)
    colA = np.zeros((n_cores, T * P), np.float16)
    wA = np.zeros((n_cores, T * P), np.float16)
    gsrcA = np.zeros((n_cores, T * P), np.int64)  # debug/emulation only
    idxA[s_core, pos] = s_lidx.astype(np.int16)
    colA[s_core, pos] = s_col.astype(np.float16)
    wA[s_core, pos] = s_w.astype(np.float16)
    gsrcA[s_core, pos] = src[order]

    # gather index layout: position i -> partition i%16, column i//16,
    # replicated across the eight 16-partition groups
    idx16 = idxA.reshape(n_cores, T * 8, 16).transpose(0, 2, 1)
    idx128 = np.ascontiguousarray(np.tile(idx16, (1, 8, 1)))  # [NC,128,T*8]

    def tp(a):  # [T*P] stream -> [P, T] (column t = chunk t)
        return np.ascontiguousarray(a.reshape(n_cores, T, P).transpose(0, 2, 1))

    return dict(
        K=K, chunk_base=chunk_base, spans=spans, group_span=group_span,
        groups=groups, T=T, n_sub=n_sub, n_win=n_win, n_cell=n_cell,
        idx=idx128, dst=tp(colA), w=tp(wA),
        _gsrc=gsrcA, _pos_w=wA,  # for host-side emulation checks
    )


def preprocess(inputs, cfg=CFG):
    inp = {k: np.asarray(v) for k, v in inputs.items()}
    H, OUT = cfg["H"], cfg["OUT"]

    R = {
        0: _prep_relation(inp["e0_src"], inp["e0_dst"], cfg["DPC"], cfg["ND"],
                          4, SUB, cfg["CG0"]),
        1: _prep_relation(inp["e1_src"], inp["e1_dst"], cfg["DPC"], cfg["ND"],
                          4, SUB, cfg["CG0"]),
        2: _prep_relation(inp["e2_src"], inp["e2_dst"], cfg["APC"], cfg["NA"],
                          2, SUB, cfg["CG1"]),
        3: _prep_relation(inp["e3_src"], inp["e3_dst"], cfg["APC"], cfg["NB"],
                          2, SUB, cfg["CG1"]),
    }

    W0 = np.einsum("rb,bio->rio", inp["coef0"], inp["basis0"]).astype(np.float32)
    W1 = np.einsum("rb,bio->rio", inp["coef1"], inp["basis1"]).astype(np.float32)
    W2 = np.einsum("rb,bio->rio", inp["coef2"], inp["basis2"]).astype(np.float32)

    common = {
        "w00": np.ascontiguousarray(W0[0].astype(np.float16)),
        "w01": np.ascontiguousarray(W0[1].astype(np.float16)),
        "w12": np.ascontiguousarray(W1[2].astype(np.float16)),
        "w13": np.ascontiguousarray(W1[3].astype(np.float16)),
        "w20": np.ascontiguousarray(W2[0].astype(np.float16)),
        "w21": np.ascontiguousarray(W2[1].astype(np.float16)),
        "bias0t": np.ascontiguousarray(
            np.broadcast_to(inp["bias0"].astype(np.float32), (P, H))
        ),
        "bias1c": np.ascontiguousarray(inp["bias1"].astype(np.float32)[:, None]),
        "bias2t": np.ascontiguousarray(
            np.broadcast_to(inp["bias2"].astype(np.float32), (P, OUT))
        ),
        "iota2": np.ascontiguousarray(
            np.broadcast_to(np.arange(P * WPP, dtype=np.float16), (P, P * WPP))
        ),
        "feat_a16": np.ascontiguousarray(inp["feat_a"].astype(np.float16)),
        "feat_b16": np.ascontiguousarray(inp["feat_b"].astype(np.float16)),
    }

    in_maps = []
    for c in range(NCORES):
        m = dict(common)
        for r in range(4):
            m[f"r{r}_idx"] = R[r]["idx"][c]
            m[f"r{r}_dst"] = R[r]["dst"][c]
            m[f"r{r}_w"] = R[r]["w"][c]
        in_maps.append(m)

    sched = {
        r: {k: R[r][k] for k in
            ("K", "chunk_base", "spans", "group_span", "groups", "T",
             "n_sub", "n_win", "n_cell")}
        for r in R
    }
    sched["_debug"] = {r: {"gsrc": R[r]["_gsrc"], "w": R[r]["_pos_w"]} for r in R}
    return sched, in_maps


def build_program(sched, cfg=CFG, phases=("L0", "AG1", "L1a", "AG2a", "L1b",
                                          "AG2b", "L2a", "L2b"), repeat=1):
    import concourse.bass as bass
    import concourse.mybir as mybir
    import concourse.tile as tile
    from concourse import bacc, library_config

    f16 = mybir.dt.float16
    f32 = mybir.dt.float32
    i16 = mybir.dt.int16
    Alu = mybir.AluOpType
    Act = mybir.ActivationFunctionType

    H, OUT = cfg["H"], cfg["OUT"]
    CW = P * WPP
    NA_PAD = 4 * SUB          # 100352
    ND_PAD = NCORES * cfg["DPC"]   # 50176 = 2*SUB
    RG = [list(range(NCORES))]

    CHMAX = max(max(n for _, n in sched[r]["group_span"]) for r in range(4))

    nc = bacc.Bacc("TRN2", target_bir_lowering=False, debug=False,
                   num_devices=NCORES, num_swdge_queues=4)

    feat_a16 = nc.dram_tensor("feat_a16", [cfg["NA"], H], f16, kind="ExternalInput")
    feat_b16 = nc.dram_tensor("feat_b16", [cfg["NB"], H], f16, kind="ExternalInput")
    meta_d = {}
    for r in range(4):
        T = sched[r]["T"]
        meta_d[r] = dict(
            idx=nc.dram_tensor(f"r{r}_idx", [P, T * 8], i16, kind="ExternalInput"),
            dst=nc.dram_tensor(f"r{r}_dst", [P, T], f16, kind="ExternalInput"),
            w=nc.dram_tensor(f"r{r}_w", [P, T], f16, kind="ExternalInput"),
        )
    consts_spec = {
        "w00": ([H, H], f16), "w01": ([H, H], f16),
        "w12": ([H, H], f16), "w13": ([H, H], f16),
        "w20": ([H, OUT], f16), "w21": ([H, OUT], f16),
        "bias0t": ([P, H], f32), "bias1c": ([P, 1], f32),
        "bias2t": ([P, OUT], f32), "iota2": ([P, CW], f16),
    }
    const_d = {
        k: nc.dram_tensor(k, shape, dt, kind="ExternalInput")
        for k, (shape, dt) in consts_spec.items()
    }
    out_d = nc.dram_tensor("out_d", [cfg["DPC"], OUT], f32, kind="ExternalOutput")

    h1_slice = nc.dram_tensor("h1_slice", [cfg["DPC"], H], f16)
    h1_full = nc.dram_tensor("h1_full", [ND_PAD, H], f16, addr_space="Shared")
    ta_slice = nc.dram_tensor("ta_slice", [cfg["APC"], OUT], f16)
    tb_slice = nc.dram_tensor("tb_slice", [cfg["APC"], OUT], f16)
    ta_full = nc.dram_tensor("ta_full", [NA_PAD, OUT], f16, addr_space="Shared")
    tb_full = nc.dram_tensor("tb_full", [NA_PAD, OUT], f16, addr_space="Shared")
    tpad_a = nc.dram_tensor("tpad_a", [NA_PAD, H], f16)
    tpad_b = nc.dram_tensor("tpad_b", [NA_PAD, H], f16)

    n_win_d = sched[0]["n_win"]    # 49
    n_cell_d = sched[0]["n_cell"]  # 25
    groups0 = sched[0]["groups"]

    def wins_of(rel, ce):
        return [w for w in (ce * WPP, ce * WPP + 1) if w < sched[rel]["n_win"]]

    with tile.TileContext(nc) as tc, ExitStack() as ctx:
        sb = ctx.enter_context(tc.tile_pool(name="sb", bufs=1))
        ps = ctx.enter_context(tc.tile_pool(name="ps", bufs=1, space="PSUM"))

        touch_v = sb.tile([1, 1], f32, name="touch_v", tag="touch_v")
        touch_g = sb.tile([1, 1], f32, name="touch_g", tag="touch_g")

        def touch(t, engine="v"):
            eng = nc.vector if engine == "v" else nc.gpsimd
            dest = touch_v if engine == "v" else touch_g
            eng.tensor_copy(out=dest[:], in_=t[0:1, 0:1])

        cs = {}
        for k, (shape, dt) in consts_spec.items():
            t = sb.tile(shape, dt, name=f"c_{k}", tag=f"c_{k}")
            nc.sync.dma_start(out=t[:], in_=const_d[k][:, :])
            cs[k] = t
        msb = {}
        for r in range(4):
            T = sched[r]["T"]
            e = {}
            for part, dt, wid in (("idx", i16, T * 8), ("dst", f16, T),
                                  ("w", f16, T)):
                t = sb.tile([P, wid], dt, name=f"m{r}_{part}", tag=f"m{r}_{part}")
                nc.sync.dma_start(out=t[:], in_=meta_d[r][part][:, :])
                e[part] = t
            msb[r] = e

        for t in cs.values():
            touch(t)
        for r in range(4):
            touch(msb[r]["dst"])
            touch(msb[r]["w"])
            touch(msb[r]["idx"], engine="g")

        nc.gpsimd.load_library(library_config.mlp)

        MAXC = 8  # chunks per dma_gather: >1024 descriptors wedges the SWDGE
        qrr = [0]

        def gather_group(rel, g, tables):
            sch = sched[rel]
            c0g, nchg = sch["group_span"][g]
            buf = sb.tile([P, CHMAX, P], f16, name="G", tag="G", bufs=3)
            for q, tbl in enumerate(tables):
                c0, nch = sch["spans"][(g, q)]
                for s0 in range(0, nch, MAXC):
                    a = c0 + s0
                    n = min(MAXC, nch - s0)
                    nc.gpsimd.dma_gather(
                        buf[:, a - c0g:a - c0g + n, :],
                        tbl,
                        msb[rel]["idx"][:, a * 8:(a + n) * 8],
                        n * P,
                        n * P,
                        P,
                        queue_num=qrr[0],
                    )
                    qrr[0] = (qrr[0] + 1) % 4
            return buf, c0g

        def cell_chunks(rel, ce):
            sch = sched[rel]
            return [
                int(sch["chunk_base"][ce, q]) + k
                for q in range(sch["n_sub"])
                for k in range(int(sch["K"][ce, q]))
            ]

        def mask_for(rel, c):
            mk = sb.tile([P, CW], f16, name="mk", tag="mk", bufs=8)
            nc.vector.scalar_tensor_tensor(
                out=mk[:],
                in0=cs["iota2"][:],
                scalar=msb[rel]["dst"][:, c:c + 1],
                in1=msb[rel]["w"][:, c:c + 1].to_broadcast([P, CW]),
                op0=Alu.is_equal,
                op1=Alu.mult,
            )
            return mk

        def cell_agg(rel, ce, buf, c0g):
            """Segment-mean for one 256-dst cell -> PSUM [H, 256]."""
            chunks = cell_chunks(rel, ce)
            pA = ps.tile([P, CW], f32, name="pA", tag="pA", bufs=3)
            for j, c in enumerate(chunks):
                mk = mask_for(rel, c)
                nc.tensor.matmul(
                    out=pA[:], lhsT=buf[:, c - c0g, :], rhs=mk[:],
                    start=(j == 0), stop=(j == len(chunks) - 1),
                )
            return pA

        agg_store = sb.tile([P, n_win_d * P], f16, name="agg_store",
                            tag="agg_store")
        partial = sb.tile([P, n_win_d * OUT], f32, name="partial", tag="partial")

        feat_a_tbls = [feat_a16[q * SUB:min(cfg["NA"], (q + 1) * SUB), :]
                       for q in range(4)]
        feat_b_tbls = [feat_b16[q * SUB:min(cfg["NB"], (q + 1) * SUB), :]
                       for q in range(4)]
        h1_tbls = [h1_full[0:SUB, :], h1_full[SUB:2 * SUB, :]]
        tpa_tbls = [tpad_a[q * SUB:(q + 1) * SUB, :] for q in range(4)]
        tpb_tbls = [tpad_b[q * SUB:(q + 1) * SUB, :] for q in range(4)]

        def emit_iteration():
            # ---------------- Layer 0 ----------------
            with nc.named_scope("L0"):
                if "L0" in phases:
                    # pass 1: relation 0 aggregates, parked in SBUF
                    for g, cells in enumerate(groups0):
                        buf, c0g = gather_group(0, g, feat_a_tbls)
                        for ce in cells:
                            pA = cell_agg(0, ce, buf, c0g)
                            nw = len(wins_of(0, ce))
                            nc.vector.tensor_copy(
                                out=agg_store[:, ce * CW:ce * CW + nw * P],
                                in_=pA[:, 0:nw * P],
                            )
                    # pass 2: relation 1 aggregates + transform + relu
                    for g, cells in enumerate(groups0):
                        buf, c0g = gather_group(1, g, feat_b_tbls)
                        for ce in cells:
                            pA = cell_agg(1, ce, buf, c0g)
                            a1 = sb.tile([P, CW], f16, name="a1", tag="a1",
                                         bufs=3)
                            nc.vector.tensor_copy(out=a1[:], in_=pA[:])
                            for o, w in enumerate(wins_of(0, ce)):
                                pB = ps.tile([P, H], f32, name="pB", tag="pB",
                                             bufs=2)
                                nc.tensor.matmul(
                                    out=pB[:],
                                    lhsT=agg_store[:, w * P:(w + 1) * P],
                                    rhs=cs["w00"][:], start=True, stop=False)
                                nc.tensor.matmul(
                                    out=pB[:], lhsT=a1[:, o * P:(o + 1) * P],
                                    rhs=cs["w01"][:], start=False, stop=True)
                                tmp = sb.tile([P, H], f32, name="tmp", tag="tmp",
                                              bufs=3)
                                nc.vector.tensor_tensor(
                                    out=tmp[:], in0=pB[:], in1=cs["bias0t"][:],
                                    op=Alu.add)
                                h1sb = sb.tile([P, H], f16, name="h1sb",
                                               tag="h1sb", bufs=3)
                                nc.vector.tensor_scalar_max(
                                    out=h1sb[:], in0=tmp[:], scalar1=0.0)
                                nc.sync.dma_start(
                                    out=h1_slice[w * P:(w + 1) * P, :],
                                    in_=h1sb[:])

            with nc.named_scope("AG1"):
                if "AG1" in phases:
                    nc.gpsimd.collective_compute(
                        "AllGather", mybir.AluOpType.bypass, replica_groups=RG,
                        ins=[h1_slice[:, :]], outs=[h1_full[:, :]],
                    )

            # ---------------- Layer 1 (+ fused W2 transform) ----------------
            def l1_pass(rel, w1_t, w2_t, t_slice):
                for g, cells in enumerate(sched[rel]["groups"]):
                    buf, c0g = gather_group(rel, g, h1_tbls)
                    for ce in cells:
                        pA = cell_agg(rel, ce, buf, c0g)
                        a_sb = sb.tile([P, CW], f16, name="a1", tag="a1", bufs=3)
                        nc.vector.tensor_copy(out=a_sb[:], in_=pA[:])
                        for o, w in enumerate(wins_of(rel, ce)):
                            pB2 = ps.tile([P, P], f32, name="pB2", tag="pB",
                                          bufs=2)
                            nc.tensor.matmul(out=pB2[:], lhsT=w1_t[:],
                                             rhs=a_sb[:, o * P:(o + 1) * P],
                                             start=True, stop=True)
                            h2T = sb.tile([P, P], f16, name="h2T", tag="h2T",
                                          bufs=3)
                            nc.scalar.activation(out=h2T[:], in_=pB2[:],
                                                 func=Act.Relu,
                                                 bias=cs["bias1c"][:], scale=1.0)
                            pC = ps.tile([P, OUT], f32, name="pC", tag="pC",
                                         bufs=3)
                            nc.tensor.matmul(out=pC[:], lhsT=h2T[:], rhs=w2_t[:],
                                             start=True, stop=True)
                            tsb = sb.tile([P, OUT], f16, name="tsb", tag="tsb",
                                          bufs=3)
                            nc.vector.tensor_copy(out=tsb[:], in_=pC[:])
                            nc.sync.dma_start(
                                out=t_slice[w * P:(w + 1) * P, :], in_=tsb[:])

            with nc.named_scope("L1a"):
                if "L1a" in phases:
                    l1_pass(2, cs["w12"], cs["w20"], ta_slice)
            with nc.named_scope("AG2a"):
                if "AG2a" in phases:
                    nc.gpsimd.collective_compute(
                        "AllGather", mybir.AluOpType.bypass, replica_groups=RG,
                        ins=[ta_slice[:, :]], outs=[ta_full[:, :]],
                    )
            with nc.named_scope("L1b"):
                if "L1b" in phases:
                    l1_pass(3, cs["w13"], cs["w21"], tb_slice)
                    if "AG2a" in phases:
                        hh = NA_PAD // 2
                        nc.sync.dma_start(out=tpad_a[0:hh, 0:OUT],
                                          in_=ta_full[0:hh, :])
                        nc.sync.dma_start(out=tpad_a[hh:NA_PAD, 0:OUT],
                                          in_=ta_full[hh:NA_PAD, :])
            with nc.named_scope("AG2b"):
                if "AG2b" in phases:
                    nc.gpsimd.collective_compute(
                        "AllGather", mybir.AluOpType.bypass, replica_groups=RG,
                        ins=[tb_slice[:, :]], outs=[tb_full[:, :]],
                    )
                    hh = NA_PAD // 2
                    nc.sync.dma_start(out=tpad_b[0:hh, 0:OUT],
                                      in_=tb_full[0:hh, :])
                    nc.sync.dma_start(out=tpad_b[hh:NA_PAD, 0:OUT],
                                      in_=tb_full[hh:NA_PAD, :])

            # ---------------- Layer 2 ----------------
            def l2_pass(rel, tbls, first):
                for g, cells in enumerate(groups0):
                    buf, c0g = gather_group(rel, g, tbls)
                    for ce in cells:
                        chunks = cell_chunks(rel, ce)
                        wins = wins_of(rel, ce)
                        pCs = [ps.tile([P, OUT], f32, name=f"pC2_{o}",
                                       tag="pC", bufs=3) for o in
                               range(len(wins))]
                        for j, c in enumerate(chunks):
                            mk = mask_for(rel, c)
                            for o in range(len(wins)):
                                nc.tensor.matmul(
                                    out=pCs[o][:],
                                    lhsT=mk[:, o * P:(o + 1) * P],
                                    rhs=buf[:, c - c0g, 0:OUT],
                                    start=(j == 0),
                                    stop=(j == len(chunks) - 1),
                                )
                        for o, w in enumerate(wins):
                            if first:
                                nc.vector.tensor_copy(
                                    out=partial[:, w * OUT:(w + 1) * OUT],
                                    in_=pCs[o][:])
                            else:
                                t1 = sb.tile([P, OUT], f32, name="t1", tag="t1",
                                             bufs=3)
                                nc.vector.tensor_tensor(
                                    out=t1[:], in0=pCs[o][:],
                                    in1=partial[:, w * OUT:(w + 1) * OUT],
                                    op=Alu.add)
                                osb = sb.tile([P, OUT], f32, name="osb",
                                              tag="osb", bufs=3)
                                nc.vector.tensor_tensor(
                                    out=osb[:], in0=t1[:], in1=cs["bias2t"][:],
                                    op=Alu.add)
                                nc.sync.dma_start(
                                    out=out_d[w * P:(w + 1) * P, :], in_=osb[:])

            with nc.named_scope("L2a"):
                if "L2a" in phases:
                    l2_pass(0, tpa_tbls, True)
            with nc.named_scope("L2b"):
                if "L2b" in phases:
                    l2_pass(1, tpb_tbls, False)

        for _rep in range(repeat):
            emit_iteration()

    return nc


LAST_RESULTS = None


def kernel(**inputs):
    global LAST_RESULTS
    from concourse.bass_utils import run_bass_kernel_spmd

    sched, in_maps = preprocess(inputs, CFG)
    nc = build_program(sched, CFG)
    nc.finalize()
    res = run_bass_kernel_spmd(nc, in_maps, list(range(NCORES)), trace=False)
    LAST_RESULTS = res
    out = np.concatenate([res.results[c]["out_d"] for c in range(NCORES)], axis=0)
    return np.ascontiguousarray(out[:CFG["ND"]].astype(np.float32))

